# revision 22
# baseline (speedup 1.0000x reference)
"""BitNet linear layer (b1.58-style) on 8 Trainium2 NeuronCores.

Computes: scale = 1e-4 + mean(|W|); q = clip(round(W/scale), -1, 1);
          out = scale * (x @ q.T)
for x [4, 2048, 2048] f32 and W [8192, 2048] f32.

Sharding: tensor-parallel over out_features. Each core gets the full x
(replicated) and a 1024-row shard of the ternary q; cores run fully
independently and the host concatenates the per-core [8192, 1024]
output slices along the feature axis.

The elementwise prep runs once on the host (it is ~0.1% of the FLOPs
and would otherwise be redundantly recomputed per core): the exact
global scale and ternary q (bit-identical rounding vs the reference),
the f32->bf16 casts, and the transposes into SBUF-ready layouts.
`scale` is folded into the bf16 x cast, which is free in accuracy
terms (a single bf16 rounding either way), so the device applies no
scale at all. Remaining error is the bf16 rounding of x plus the bf16
output store (~2.2e-3 measured).

The device is then a pure gap-free bf16 matmul at the PE roofline:
2048 matmuls of N=512 at the 216 ns issue floor ~= 443 us, everything
else hidden behind it. Startup schedule (the only nontrivial part —
the SBUF-write fabric and per-queue DMA rates bound how fast q and the
first x tiles can land):

  - q ships as raw fp8 (ternary is exact; 2 MiB instead of 4) in 2-ko
    slices, alternating between the gpsimd (SWDGE, ~200 GB/s) and sync
    (HWDGE, ~100 GB/s) queues, and is expanded fp8 -> bf16 in SBUF by
    the DVE and ACT engines alternately (~1.3 us/slice each, idle at
    that point). The matmul reads bf16: an fp8 moving operand would
    stream ~20% slower (259 vs 216 ns/MM measured).
  - m-tiles 0 and 1 are interleaved ko-major so each q slice is
    consumed twice per arrival (0.86 us/ko consumption vs ~0.65 us/ko
    feed) — without this the PE stalls ~8 us on q arrival. Their x
    tiles ride the gpsimd queue in 512-col chunks woven between the
    q slices in need order.
  - ~10 dummy matmuls on an uninitialized SBUF tile (output never
    read) run during the preamble to carry the PE through the HAM
    SHORT window, so real matmuls start at 2.4 GHz, not 1.2.
  - Steady-state x rides the scalar queue as 1 MiB two-m-tile pair
    DMAs (higher HWDGE efficiency than 512 KiB singles and half the
    per-tile semaphore checks).
  - Per m-tile: 16 k-steps of two 512-col accumulating matmuls into a
    psum bank pair (8 banks -> 4 m-tiles in flight), DVE drains psum
    -> bf16 out tile, stores on the sync queue in natural [M, N-shard]
    orientation. The final m-tile runs its two psum sweeps
    sequentially so the first drain+store overlaps the second sweep.
"""

import sys

sys.path.insert(0, "/opt/trn_rl_repo")

import numpy as np
import ml_dtypes

import concourse.bass as bass
import concourse.tile as tile
from concourse import bacc, mybir
from concourse.bass_utils import run_bass_kernel_spmd

F32 = mybir.dt.float32
BF16 = mybir.dt.bfloat16
FP8 = mybir.dt.float8e4
U32 = mybir.dt.uint32
BF16_NP = ml_dtypes.bfloat16
FP8_NP = ml_dtypes.float8_e4m3

NCORES = 8
M = 8192          # tokens (4*2048)
K = 2048          # in_features
N_FULL = 8192     # out_features
NS = N_FULL // NCORES  # 1024 per-core shard
P = 128
KO = K // P       # 16 k-tiles
MT = M // P       # 64 m-tiles
NPAIR = MT // 2   # x pair-DMA rows


def build_nc():
    nc = bacc.Bacc("TRN2", target_bir_lowering=False, debug=False,
                   num_devices=NCORES)
    # x rows pair*128+p hold [j, ko*128+m] -> scale*x[(2*pair+j)*128+m, ko*128+p]
    x_d = nc.dram_tensor("x", [M // 2, 2 * K], BF16, kind="ExternalInput")
    q_d = nc.dram_tensor("q", [P, KO * NS], FP8, kind="ExternalInput")
    o_d = nc.dram_tensor("out", [M, NS], BF16, kind="ExternalOutput")
    x_ap, q_ap, o_ap = x_d.ap(), q_d.ap(), o_d.ap()

    with tile.TileContext(nc) as tc:
        with (
            tc.tile_pool(name="qpool", bufs=1) as qpool,
            tc.tile_pool(name="xspool", bufs=4) as xspool,
            tc.tile_pool(name="xppool", bufs=3) as xppool,
            tc.tile_pool(name="opool", bufs=4) as opool,
            tc.tile_pool(name="psum_o", bufs=8, space="PSUM") as psum_o,
        ):
            # q stays fp8 in SBUF: the ternary weights are exact in fp8,
            # the PE upconverts both operands internally, and an fp8
            # moving operand streams at the same 216 ns/MM as bf16 (an
            # earlier 259 ns measurement was a chip P0 2.0 GHz window,
            # not an fp8 property). Half the SBUF footprint and half the
            # startup-burst bytes, and no expansion ops at all.
            tile_q = qpool.tile([P, KO * NS], FP8, name="q")
            warm = qpool.tile([P, 640], BF16, name="warm")

            # ---- PE warmup (HAM) --------------------------------------
            wz = warm[:].bitcast(U32)
            nc.vector.tensor_scalar(wz, wz, 0, None,
                                    mybir.AluOpType.bitwise_and)
            # dummies bridge the PE from the preamble (~7us) to when the
            # startup burst lands the first q+x slices (~10us): staying
            # busy keeps HAM at 2.4 GHz for the real matmuls.
            psW = psum_o.tile([P, 512], F32, name="psW", tag="ps")
            for _ in range(8):
                nc.tensor.matmul(psW[:], lhsT=warm[:, 0:P],
                                 rhs=warm[:, P:640], start=True, stop=True)

            # ---- q slice loads (all on the gpsimd/SWDGE queue, the
            # fastest for these strided 128-partition tiles) ------------
            def q_dma(g):
                nc.gpsimd.dma_start(
                    tile_q[:, 2 * g * NS:2 * (g + 1) * NS],
                    q_ap[:, 2 * g * NS:2 * (g + 1) * NS])

            # x singles for m-tiles 0-3 (pair row mt//2, column half mt%2)
            def xs_dma(mt, chunks, eng):
                xt = xspool.tile([P, K], BF16, name=f"x_{mt}", tag="x")
                r0, c0 = (mt // 2) * P, (mt % 2) * K
                step = K // chunks
                for c in range(chunks):
                    eng.dma_start(
                        xt[:, c * step:(c + 1) * step],
                        x_ap[r0:r0 + P, c0 + c * step:c0 + (c + 1) * step])
                return xt

            xt2 = xs_dma(2, 1, nc.scalar)
            xt3 = xs_dma(3, 1, nc.scalar)

            # x0/x1 chunks woven between gpsimd q slices in need order
            xt0 = xspool.tile([P, K], BF16, name="x_0", tag="x")
            xt1 = xspool.tile([P, K], BF16, name="x_1", tag="x")
            def x01_chunk(xt, mt, c):
                nc.gpsimd.dma_start(
                    xt[:, c * 512:(c + 1) * 512],
                    x_ap[0:P, mt * K + c * 512:mt * K + (c + 1) * 512])

            q_dma(0)
            x01_chunk(xt0, 0, 0); x01_chunk(xt1, 1, 0)
            q_dma(1)
            x01_chunk(xt0, 0, 1); x01_chunk(xt1, 1, 1)
            q_dma(2)
            x01_chunk(xt0, 0, 2); x01_chunk(xt1, 1, 2)
            q_dma(3)
            x01_chunk(xt0, 0, 3); x01_chunk(xt1, 1, 3)
            for g in range(4, KO // 2):
                q_dma(g)

            # ---- main loop: out[m, n] = sum_k x[m,k] q[n,k] -----------
            def mm_pair(ps2, xt, base, ko):
                nc.tensor.matmul(
                    ps2[0][:], lhsT=xt[:, base + ko * P:base + (ko + 1) * P],
                    rhs=tile_q[:, ko * NS:ko * NS + 512],
                    start=(ko == 0), stop=(ko == KO - 1))
                nc.tensor.matmul(
                    ps2[1][:], lhsT=xt[:, base + ko * P:base + (ko + 1) * P],
                    rhs=tile_q[:, ko * NS + 512:(ko + 1) * NS],
                    start=(ko == 0), stop=(ko == KO - 1))

            def drain_store(mt, ps2):
                ot = opool.tile([P, NS], BF16, name=f"o_{mt}", tag="o")
                nc.vector.tensor_scalar(
                    ot[:, 0:512], ps2[0][:], 1.0, None, mybir.AluOpType.mult)
                nc.vector.tensor_scalar(
                    ot[:, 512:1024], ps2[1][:], 1.0, None,
                    mybir.AluOpType.mult)
                nc.sync.dma_start(o_ap[mt * P:(mt + 1) * P, :], ot[:])

            def ps_pair(mt):
                return (psum_o.tile([P, 512], F32, name=f"psA_{mt}", tag="ps"),
                        psum_o.tile([P, 512], F32, name=f"psB_{mt}", tag="ps"))

            # m-tiles 0/1 interleaved ko-major (see header)
            ps0, ps1 = ps_pair(0), ps_pair(1)
            for ko in range(KO):
                mm_pair(ps0, xt0, 0, ko)
                mm_pair(ps1, xt1, 0, ko)
            drain_store(0, ps0)
            drain_store(1, ps1)

            for mt, xt in ((2, xt2), (3, xt3)):
                ps = ps_pair(mt)
                for ko in range(KO):
                    mm_pair(ps, xt, 0, ko)
                drain_store(mt, ps)

            # steady state: 1 MiB pair DMAs; both m-tiles of a pair run
            # ko-major interleaved (4 psum banks) so there is only one
            # semaphore-check hiccup (~50ns) per pair instead of two
            for pair in range(2, NPAIR - 1):
                xt = xppool.tile([P, 2 * K], BF16, name=f"xp_{pair}",
                                 tag="xp")
                nc.scalar.dma_start(xt[:], x_ap[pair * P:(pair + 1) * P, :])
                psj = (ps_pair(2 * pair), ps_pair(2 * pair + 1))
                for ko in range(KO):
                    mm_pair(psj[0], xt, 0, ko)
                    mm_pair(psj[1], xt, K, ko)
                drain_store(2 * pair, psj[0])
                drain_store(2 * pair + 1, psj[1])

            # last pair: sequential m-tiles, and the final m-tile runs
            # its two psum sweeps back to back so the first drain+store
            # overlaps the second sweep (shorter tail)
            for pair in range(NPAIR - 1, NPAIR):
                xt = xppool.tile([P, 2 * K], BF16, name=f"xp_{pair}",
                                 tag="xp")
                nc.scalar.dma_start(xt[:], x_ap[pair * P:(pair + 1) * P, :])
                for j in range(2):
                    mt = 2 * pair + j
                    if mt < MT - 1:
                        ps = ps_pair(mt)
                        for ko in range(KO):
                            mm_pair(ps, xt, j * K, ko)
                        drain_store(mt, ps)
                    else:
                        # last m-tile: sequential psum sweeps so the
                        # first drain+store overlaps the second sweep
                        psA, psB = ps_pair(mt)
                        ot = opool.tile([P, NS], BF16, name=f"o_{mt}",
                                        tag="o")
                        for ko in range(KO):
                            nc.tensor.matmul(
                                psA[:],
                                lhsT=xt[:, j * K + ko * P:
                                        j * K + (ko + 1) * P],
                                rhs=tile_q[:, ko * NS:ko * NS + 512],
                                start=(ko == 0), stop=(ko == KO - 1))
                        nc.vector.tensor_scalar(
                            ot[:, 0:512], psA[:], 1.0, None,
                            mybir.AluOpType.mult)
                        nc.sync.dma_start(
                            o_ap[mt * P:(mt + 1) * P, 0:512], ot[:, 0:512])
                        for ko in range(KO):
                            nc.tensor.matmul(
                                psB[:],
                                lhsT=xt[:, j * K + ko * P:
                                        j * K + (ko + 1) * P],
                                rhs=tile_q[:, ko * NS + 512:(ko + 1) * NS],
                                start=(ko == 0), stop=(ko == KO - 1))
                        nc.vector.tensor_scalar(
                            ot[:, 512:1024], psB[:], 1.0, None,
                            mybir.AluOpType.mult)
                        nc.sync.dma_start(
                            o_ap[mt * P:(mt + 1) * P, 512:1024],
                            ot[:, 512:1024])

    nc.compile()
    return nc


_NC_CACHE = None


def get_nc():
    global _NC_CACHE
    if _NC_CACHE is None:
        _NC_CACHE = build_nc()
    return _NC_CACHE


def make_in_maps(x, weight):
    x2 = np.asarray(x, dtype=np.float32).reshape(M, K)
    w = np.asarray(weight, dtype=np.float32)

    # exact reference prep: scale from the full W, ternary q
    scale = np.float32(1e-4) + np.abs(w).mean(dtype=np.float32)
    q = np.clip(np.rint(w / scale), -1.0, 1.0).astype(np.float32)

    # xdev[pair*128+p, j*2048 + ko*128+m] = scale*x[(2*pair+j)*128+m, ko*128+p]
    xs = (x2 * scale).reshape(NPAIR, 2, P, KO, P)  # [pair, j, m, ko, p]
    xdev = np.ascontiguousarray(
        xs.transpose(0, 4, 1, 3, 2).reshape(M // 2, 2 * K).astype(BF16_NP))

    # qdev_c[p, ko*1024+n] = q[c*1024+n, ko*128+p]  (ternary: exact in fp8)
    q4 = q.reshape(NCORES, NS, KO, P).transpose(0, 3, 2, 1)  # [c, p, ko, n]
    qdev = np.ascontiguousarray(q4.reshape(NCORES, P, KO * NS).astype(FP8_NP))

    return [{"x": xdev, "q": qdev[c]} for c in range(NCORES)]


def kernel(x, weight):
    nc = get_nc()
    in_maps = make_in_maps(x, weight)
    try:
        res = run_bass_kernel_spmd(nc, in_maps, list(range(NCORES)))
    except Exception:
        # transient device errors have been observed on first touch; retry once
        res = run_bass_kernel_spmd(nc, in_maps, list(range(NCORES)))
    out = np.concatenate(
        [np.asarray(res.results[c]["out"]) for c in range(NCORES)], axis=1)
    return np.ascontiguousarray(out, dtype=np.float32).reshape(4, 2048, N_FULL)


# revision 24
# speedup vs baseline: 1.0212x; 1.0212x over previous
"""BitNet linear layer (b1.58-style) on 8 Trainium2 NeuronCores.

Computes: scale = 1e-4 + mean(|W|); q = clip(round(W/scale), -1, 1);
          out = scale * (x @ q.T)
for x [4, 2048, 2048] f32 and W [8192, 2048] f32.

Sharding: tensor-parallel over out_features. Each core gets the full x
(replicated) and a 1024-row shard of the ternary q; cores run fully
independently and the host concatenates the per-core [8192, 1024]
output slices along the feature axis.

The elementwise prep runs once on the host (it is ~0.1% of the FLOPs
and would otherwise be redundantly recomputed per core): the exact
global scale and ternary q (bit-identical rounding vs the reference),
the f32->bf16 casts, and the transposes into SBUF-ready layouts.
`scale` is folded into the bf16 x cast, which is free in accuracy
terms (a single bf16 rounding either way), so the device applies no
scale at all. Remaining error is the bf16 rounding of x plus the bf16
output store (~2.2e-3 measured).

The device is then a pure gap-free bf16 matmul at the PE roofline:
2048 matmuls of N=512 at the 216 ns issue floor ~= 443 us, everything
else hidden behind it. Startup schedule (the only nontrivial part —
the SBUF-write fabric and per-queue DMA rates bound how fast q and the
first x tiles can land):

  - q ships as raw fp8 (ternary is exact; 2 MiB instead of 4) in 2-ko
    slices, alternating between the gpsimd (SWDGE, ~200 GB/s) and sync
    (HWDGE, ~100 GB/s) queues, and is expanded fp8 -> bf16 in SBUF by
    the DVE and ACT engines alternately (~1.3 us/slice each, idle at
    that point). The matmul reads bf16: an fp8 moving operand would
    stream ~20% slower (259 vs 216 ns/MM measured).
  - m-tiles 0 and 1 are interleaved ko-major so each q slice is
    consumed twice per arrival (0.86 us/ko consumption vs ~0.65 us/ko
    feed) — without this the PE stalls ~8 us on q arrival. Their x
    tiles ride the gpsimd queue in 512-col chunks woven between the
    q slices in need order.
  - ~10 dummy matmuls on an uninitialized SBUF tile (output never
    read) run during the preamble to carry the PE through the HAM
    SHORT window, so real matmuls start at 2.4 GHz, not 1.2.
  - Steady-state x rides the scalar queue as 1 MiB two-m-tile pair
    DMAs (higher HWDGE efficiency than 512 KiB singles and half the
    per-tile semaphore checks).
  - Per m-tile: 16 k-steps of two 512-col accumulating matmuls into a
    psum bank pair (8 banks -> 4 m-tiles in flight), DVE drains psum
    -> bf16 out tile, stores on the sync queue in natural [M, N-shard]
    orientation. The final m-tile runs its two psum sweeps
    sequentially so the first drain+store overlaps the second sweep.
"""

import sys

sys.path.insert(0, "/opt/trn_rl_repo")

import numpy as np
import ml_dtypes

import concourse.bass as bass
import concourse.tile as tile
from concourse import bacc, mybir
from concourse.bass_utils import run_bass_kernel_spmd

F32 = mybir.dt.float32
BF16 = mybir.dt.bfloat16
FP8 = mybir.dt.float8e4
U32 = mybir.dt.uint32
BF16_NP = ml_dtypes.bfloat16
FP8_NP = ml_dtypes.float8_e4m3

NCORES = 8
M = 8192          # tokens (4*2048)
K = 2048          # in_features
N_FULL = 8192     # out_features
NS = N_FULL // NCORES  # 1024 per-core shard
P = 128
KO = K // P       # 16 k-tiles
MT = M // P       # 64 m-tiles
NPAIR = MT // 2   # x pair-DMA rows


def build_nc():
    nc = bacc.Bacc("TRN2", target_bir_lowering=False, debug=False,
                   num_devices=NCORES)
    # x rows pair*128+p hold [j, ko*128+m] -> scale*x[(2*pair+j)*128+m, ko*128+p]
    x_d = nc.dram_tensor("x", [M // 2, 2 * K], BF16, kind="ExternalInput")
    q_d = nc.dram_tensor("q", [P, KO * NS], FP8, kind="ExternalInput")
    o_d = nc.dram_tensor("out", [M, NS], BF16, kind="ExternalOutput")
    x_ap, q_ap, o_ap = x_d.ap(), q_d.ap(), o_d.ap()

    with tile.TileContext(nc) as tc:
        with (
            tc.tile_pool(name="qpool", bufs=1) as qpool,
            tc.tile_pool(name="xspool", bufs=4) as xspool,
            tc.tile_pool(name="xppool", bufs=3) as xppool,
            tc.tile_pool(name="opool", bufs=4) as opool,
            tc.tile_pool(name="psum_o", bufs=8, space="PSUM") as psum_o,
        ):
            # q stays fp8 in SBUF: the ternary weights are exact in fp8,
            # the PE upconverts both operands internally, and an fp8
            # moving operand streams at the same 216 ns/MM as bf16 (an
            # earlier 259 ns measurement was a chip P0 2.0 GHz window,
            # not an fp8 property). Half the SBUF footprint and half the
            # startup-burst bytes, and no expansion ops at all.
            tile_q = qpool.tile([P, KO * NS], FP8, name="q")
            warm = qpool.tile([P, 640], BF16, name="warm")

            # ---- PE warmup (HAM) --------------------------------------
            wz = warm[:].bitcast(U32)
            nc.vector.tensor_scalar(wz, wz, 0, None,
                                    mybir.AluOpType.bitwise_and)
            # dummies bridge the PE from the preamble (~7us) to when the
            # startup burst lands the first q+x slices (~10us): staying
            # busy keeps HAM at 2.4 GHz for the real matmuls.
            psW = psum_o.tile([P, 512], F32, name="psW", tag="ps")
            for _ in range(14):
                nc.tensor.matmul(psW[:], lhsT=warm[:, 0:P],
                                 rhs=warm[:, P:640], start=True, stop=True)

            # ---- startup burst ----------------------------------------
            # Everything rides the gpsimd/SWDGE queue (the fastest for
            # these strided 128-partition tiles) in strict need order —
            # program order on one queue is also a natural throttle, so
            # no later prefetch can steal bandwidth from the critical
            # first slices. q6/q7 go to the sync queue (idle until the
            # first out-store at ~29us): slow but early enough. Every
            # queue shows a ~12-13us first-completion floor regardless
            # of size, so T0 ~= 14us is the data-bound start.
            def q_dma(g, eng):
                eng.dma_start(
                    tile_q[:, 2 * g * NS:2 * (g + 1) * NS],
                    q_ap[:, 2 * g * NS:2 * (g + 1) * NS])

            # x singles for m-tiles 0-3 (pair row mt//2, column half mt%2)
            def xs_dma(mt, chunks, eng):
                xt = xspool.tile([P, K], BF16, name=f"x_{mt}", tag="x")
                r0, c0 = (mt // 2) * P, (mt % 2) * K
                step = K // chunks
                for c in range(chunks):
                    eng.dma_start(
                        xt[:, c * step:(c + 1) * step],
                        x_ap[r0:r0 + P, c0 + c * step:c0 + (c + 1) * step])
                return xt

            # x0/x1 chunks woven between gpsimd q slices in need order
            xt0 = xspool.tile([P, K], BF16, name="x_0", tag="x")
            xt1 = xspool.tile([P, K], BF16, name="x_1", tag="x")
            def x01_chunk(xt, mt, c):
                nc.gpsimd.dma_start(
                    xt[:, c * 512:(c + 1) * 512],
                    x_ap[0:P, mt * K + c * 512:mt * K + (c + 1) * 512])

            q_dma(6, nc.sync)
            q_dma(7, nc.sync)
            q_dma(0, nc.gpsimd)
            x01_chunk(xt0, 0, 0); x01_chunk(xt1, 1, 0)
            q_dma(1, nc.gpsimd)
            q_dma(2, nc.gpsimd)
            x01_chunk(xt0, 0, 1); x01_chunk(xt1, 1, 1)
            q_dma(3, nc.gpsimd)
            q_dma(4, nc.gpsimd)
            x01_chunk(xt0, 0, 2); x01_chunk(xt1, 1, 2)
            q_dma(5, nc.gpsimd)
            x01_chunk(xt0, 0, 3); x01_chunk(xt1, 1, 3)
            xt2 = xs_dma(2, 1, nc.gpsimd)
            xt3 = xs_dma(3, 1, nc.gpsimd)

            # ---- main loop: out[m, n] = sum_k x[m,k] q[n,k] -----------
            def mm_pair(ps2, xt, base, ko):
                nc.tensor.matmul(
                    ps2[0][:], lhsT=xt[:, base + ko * P:base + (ko + 1) * P],
                    rhs=tile_q[:, ko * NS:ko * NS + 512],
                    start=(ko == 0), stop=(ko == KO - 1))
                nc.tensor.matmul(
                    ps2[1][:], lhsT=xt[:, base + ko * P:base + (ko + 1) * P],
                    rhs=tile_q[:, ko * NS + 512:(ko + 1) * NS],
                    start=(ko == 0), stop=(ko == KO - 1))

            def drain_store(mt, ps2):
                ot = opool.tile([P, NS], BF16, name=f"o_{mt}", tag="o")
                nc.vector.tensor_scalar(
                    ot[:, 0:512], ps2[0][:], 1.0, None, mybir.AluOpType.mult)
                nc.vector.tensor_scalar(
                    ot[:, 512:1024], ps2[1][:], 1.0, None,
                    mybir.AluOpType.mult)
                nc.sync.dma_start(o_ap[mt * P:(mt + 1) * P, :], ot[:])

            def ps_pair(mt):
                return (psum_o.tile([P, 512], F32, name=f"psA_{mt}", tag="ps"),
                        psum_o.tile([P, 512], F32, name=f"psB_{mt}", tag="ps"))

            # m-tiles 0/1 interleaved ko-major (see header)
            ps0, ps1 = ps_pair(0), ps_pair(1)
            for ko in range(KO):
                mm_pair(ps0, xt0, 0, ko)
                mm_pair(ps1, xt1, 0, ko)
            drain_store(0, ps0)
            drain_store(1, ps1)

            for mt, xt in ((2, xt2), (3, xt3)):
                ps = ps_pair(mt)
                for ko in range(KO):
                    mm_pair(ps, xt, 0, ko)
                drain_store(mt, ps)

            # steady state: 1 MiB pair DMAs; both m-tiles of a pair run
            # ko-major interleaved (4 psum banks) so there is only one
            # semaphore-check hiccup (~50ns) per pair instead of two
            for pair in range(2, NPAIR - 1):
                xt = xppool.tile([P, 2 * K], BF16, name=f"xp_{pair}",
                                 tag="xp")
                nc.gpsimd.dma_start(xt[:], x_ap[pair * P:(pair + 1) * P, :])
                psj = (ps_pair(2 * pair), ps_pair(2 * pair + 1))
                for ko in range(KO):
                    mm_pair(psj[0], xt, 0, ko)
                    mm_pair(psj[1], xt, K, ko)
                drain_store(2 * pair, psj[0])
                drain_store(2 * pair + 1, psj[1])

            # last pair: sequential m-tiles, and the final m-tile runs
            # its two psum sweeps back to back so the first drain+store
            # overlaps the second sweep (shorter tail)
            for pair in range(NPAIR - 1, NPAIR):
                xt = xppool.tile([P, 2 * K], BF16, name=f"xp_{pair}",
                                 tag="xp")
                nc.gpsimd.dma_start(xt[:], x_ap[pair * P:(pair + 1) * P, :])
                for j in range(2):
                    mt = 2 * pair + j
                    if mt < MT - 1:
                        ps = ps_pair(mt)
                        for ko in range(KO):
                            mm_pair(ps, xt, j * K, ko)
                        drain_store(mt, ps)
                    else:
                        # last m-tile: sequential psum sweeps so the
                        # first drain+store overlaps the second sweep
                        psA, psB = ps_pair(mt)
                        ot = opool.tile([P, NS], BF16, name=f"o_{mt}",
                                        tag="o")
                        for ko in range(KO):
                            nc.tensor.matmul(
                                psA[:],
                                lhsT=xt[:, j * K + ko * P:
                                        j * K + (ko + 1) * P],
                                rhs=tile_q[:, ko * NS:ko * NS + 512],
                                start=(ko == 0), stop=(ko == KO - 1))
                        nc.vector.tensor_scalar(
                            ot[:, 0:512], psA[:], 1.0, None,
                            mybir.AluOpType.mult)
                        nc.sync.dma_start(
                            o_ap[mt * P:(mt + 1) * P, 0:512], ot[:, 0:512])
                        for ko in range(KO):
                            nc.tensor.matmul(
                                psB[:],
                                lhsT=xt[:, j * K + ko * P:
                                        j * K + (ko + 1) * P],
                                rhs=tile_q[:, ko * NS + 512:(ko + 1) * NS],
                                start=(ko == 0), stop=(ko == KO - 1))
                        nc.vector.tensor_scalar(
                            ot[:, 512:1024], psB[:], 1.0, None,
                            mybir.AluOpType.mult)
                        nc.sync.dma_start(
                            o_ap[mt * P:(mt + 1) * P, 512:1024],
                            ot[:, 512:1024])

    nc.compile()
    return nc


_NC_CACHE = None


def get_nc():
    global _NC_CACHE
    if _NC_CACHE is None:
        _NC_CACHE = build_nc()
    return _NC_CACHE


def make_in_maps(x, weight):
    x2 = np.asarray(x, dtype=np.float32).reshape(M, K)
    w = np.asarray(weight, dtype=np.float32)

    # exact reference prep: scale from the full W, ternary q
    scale = np.float32(1e-4) + np.abs(w).mean(dtype=np.float32)
    q = np.clip(np.rint(w / scale), -1.0, 1.0).astype(np.float32)

    # xdev[pair*128+p, j*2048 + ko*128+m] = scale*x[(2*pair+j)*128+m, ko*128+p]
    xs = (x2 * scale).reshape(NPAIR, 2, P, KO, P)  # [pair, j, m, ko, p]
    xdev = np.ascontiguousarray(
        xs.transpose(0, 4, 1, 3, 2).reshape(M // 2, 2 * K).astype(BF16_NP))

    # qdev_c[p, ko*1024+n] = q[c*1024+n, ko*128+p]  (ternary: exact in fp8)
    q4 = q.reshape(NCORES, NS, KO, P).transpose(0, 3, 2, 1)  # [c, p, ko, n]
    qdev = np.ascontiguousarray(q4.reshape(NCORES, P, KO * NS).astype(FP8_NP))

    return [{"x": xdev, "q": qdev[c]} for c in range(NCORES)]


def kernel(x, weight):
    nc = get_nc()
    in_maps = make_in_maps(x, weight)
    try:
        res = run_bass_kernel_spmd(nc, in_maps, list(range(NCORES)))
    except Exception:
        # transient device errors have been observed on first touch; retry once
        res = run_bass_kernel_spmd(nc, in_maps, list(range(NCORES)))
    out = np.concatenate(
        [np.asarray(res.results[c]["out"]) for c in range(NCORES)], axis=1)
    return np.ascontiguousarray(out, dtype=np.float32).reshape(4, 2048, N_FULL)


# revision 27
# speedup vs baseline: 1.0214x; 1.0002x over previous
"""BitNet linear layer (b1.58-style) on 8 Trainium2 NeuronCores.

Computes: scale = 1e-4 + mean(|W|); q = clip(round(W/scale), -1, 1);
          out = scale * (x @ q.T)
for x [4, 2048, 2048] f32 and W [8192, 2048] f32.

Sharding: tensor-parallel over out_features. Each core gets the full x
(replicated) and a 1024-row shard of the ternary q; cores run fully
independently and the host concatenates the per-core [8192, 1024]
output slices along the feature axis.

The elementwise prep runs once on the host (it is ~0.1% of the FLOPs
and would otherwise be redundantly recomputed per core): the exact
global scale and ternary q (bit-identical rounding vs the reference),
the f32->bf16 casts, and the transposes into SBUF-ready layouts.
`scale` is folded into the bf16 x cast, which is free in accuracy
terms (a single bf16 rounding either way), so the device applies no
scale at all. Remaining error is the bf16 rounding of x plus the bf16
output store (~2.2e-3 measured).

The device is then a pure gap-free bf16 matmul at the PE roofline:
2048 matmuls of N=512 at the 216 ns issue floor ~= 443 us, everything
else hidden behind it. Startup schedule (the only nontrivial part —
the SBUF-write fabric and per-queue DMA rates bound how fast q and the
first x tiles can land):

  - q ships AND stays fp8 (ternary is exact in fp8; 2 MiB instead of
    4, half the SBUF footprint): the matmul takes a bf16 stationary x
    against an fp8 moving q at the same 216 ns/MM cadence — the PE
    upconverts both operands internally. (An earlier "fp8 is 20%
    slower" measurement was a chip-wide P0 2.0 GHz power-state window,
    not an fp8 property; always classify runs by their steady MM
    issue delta before comparing.)
  - The whole startup burst (q slices + the first x tiles, in strict
    need order) rides the gpsimd/SWDGE queue, which is ~2x faster
    than the HWDGE queues for these strided 128-partition tiles.
    One queue in need order is also a natural throttle: later
    prefetches cannot steal bandwidth from the critical first slices
    (a 3-tile-deep x prefetch on its own queue measurably starved q).
    q6/q7 go to the sync queue, idle until the first out-store.
    Every queue shows a ~12-13 us first-completion floor regardless
    of transfer size, so real work is data-bound to start ~14 us in.
  - m-tiles 0 and 1 are interleaved ko-major so each q slice is
    consumed twice per arrival — without this the PE stalls ~5-8 us
    on q arrival even on the fast queue.
  - ~17 dummy matmuls on a zeroed SBUF tile (output never read) run
    during the preamble to carry the PE through the HAM SHORT window
    and up to the data floor, so real matmuls start at 2.4 GHz (the
    idle/cold default is 1.2 GHz).
  - Steady-state x: 1 MiB two-m-tile pair DMAs, also on gpsimd,
    emitted after the burst; the two m-tiles of a pair run ko-major
    interleaved (4 psum banks) so there is one semaphore-check hiccup
    (~50 ns) per pair instead of two.
  - Per m-tile: 16 k-steps of two 512-col accumulating matmuls into a
    psum bank pair (8 banks -> 4 m-tiles in flight), DVE drains psum
    -> bf16 out tile, stores on the sync queue in natural [M, N-shard]
    orientation. The final m-tile runs its two psum sweeps
    sequentially so the first drain+store overlaps the second sweep.
"""

import sys

sys.path.insert(0, "/opt/trn_rl_repo")

import numpy as np
import ml_dtypes

import concourse.bass as bass
import concourse.tile as tile
from concourse import bacc, mybir
from concourse.bass_utils import run_bass_kernel_spmd

F32 = mybir.dt.float32
BF16 = mybir.dt.bfloat16
FP8 = mybir.dt.float8e4
U32 = mybir.dt.uint32
BF16_NP = ml_dtypes.bfloat16
FP8_NP = ml_dtypes.float8_e4m3

NCORES = 8
M = 8192          # tokens (4*2048)
K = 2048          # in_features
N_FULL = 8192     # out_features
NS = N_FULL // NCORES  # 1024 per-core shard
P = 128
KO = K // P       # 16 k-tiles
MT = M // P       # 64 m-tiles
NPAIR = MT // 2   # x pair-DMA rows


def build_nc():
    nc = bacc.Bacc("TRN2", target_bir_lowering=False, debug=False,
                   num_devices=NCORES)
    # x rows pair*128+p hold [j, ko*128+m] -> scale*x[(2*pair+j)*128+m, ko*128+p]
    x_d = nc.dram_tensor("x", [M // 2, 2 * K], BF16, kind="ExternalInput")
    q_d = nc.dram_tensor("q", [P, KO * NS], FP8, kind="ExternalInput")
    o_d = nc.dram_tensor("out", [M, NS], BF16, kind="ExternalOutput")
    x_ap, q_ap, o_ap = x_d.ap(), q_d.ap(), o_d.ap()

    with tile.TileContext(nc) as tc:
        with (
            tc.tile_pool(name="qpool", bufs=1) as qpool,
            tc.tile_pool(name="xspool", bufs=4) as xspool,
            tc.tile_pool(name="xppool", bufs=3) as xppool,
            tc.tile_pool(name="opool", bufs=4) as opool,
            tc.tile_pool(name="psum_o", bufs=8, space="PSUM") as psum_o,
        ):
            # q stays fp8 in SBUF: the ternary weights are exact in fp8,
            # the PE upconverts both operands internally, and an fp8
            # moving operand streams at the same 216 ns/MM as bf16 (an
            # earlier 259 ns measurement was a chip P0 2.0 GHz window,
            # not an fp8 property). Half the SBUF footprint and half the
            # startup-burst bytes, and no expansion ops at all.
            tile_q = qpool.tile([P, KO * NS], FP8, name="q")
            warm = qpool.tile([P, 640], BF16, name="warm")

            # ---- PE warmup (HAM) --------------------------------------
            wz = warm[:].bitcast(U32)
            nc.vector.tensor_scalar(wz, wz, 0, None,
                                    mybir.AluOpType.bitwise_and)
            # dummies bridge the PE from the preamble (~7us) to when the
            # startup burst lands the first q+x slices (~10us): staying
            # busy keeps HAM at 2.4 GHz for the real matmuls.
            psW = psum_o.tile([P, 512], F32, name="psW", tag="ps")
            for _ in range(17):
                nc.tensor.matmul(psW[:], lhsT=warm[:, 0:P],
                                 rhs=warm[:, P:640], start=True, stop=True)

            # ---- startup burst ----------------------------------------
            # Everything rides the gpsimd/SWDGE queue (the fastest for
            # these strided 128-partition tiles) in strict need order —
            # program order on one queue is also a natural throttle, so
            # no later prefetch can steal bandwidth from the critical
            # first slices. q6/q7 go to the sync queue (idle until the
            # first out-store at ~29us): slow but early enough. Every
            # queue shows a ~12-13us first-completion floor regardless
            # of size, so T0 ~= 14us is the data-bound start.
            def q_dma(g, eng):
                eng.dma_start(
                    tile_q[:, 2 * g * NS:2 * (g + 1) * NS],
                    q_ap[:, 2 * g * NS:2 * (g + 1) * NS])

            # x singles for m-tiles 0-3 (pair row mt//2, column half mt%2)
            def xs_dma(mt, chunks, eng):
                xt = xspool.tile([P, K], BF16, name=f"x_{mt}", tag="x")
                r0, c0 = (mt // 2) * P, (mt % 2) * K
                step = K // chunks
                for c in range(chunks):
                    eng.dma_start(
                        xt[:, c * step:(c + 1) * step],
                        x_ap[r0:r0 + P, c0 + c * step:c0 + (c + 1) * step])
                return xt

            # x0/x1 chunks woven between gpsimd q slices in need order
            xt0 = xspool.tile([P, K], BF16, name="x_0", tag="x")
            xt1 = xspool.tile([P, K], BF16, name="x_1", tag="x")
            def x01_chunk(xt, mt, c):
                nc.gpsimd.dma_start(
                    xt[:, c * 512:(c + 1) * 512],
                    x_ap[0:P, mt * K + c * 512:mt * K + (c + 1) * 512])

            q_dma(6, nc.sync)
            q_dma(7, nc.sync)
            # first slice in 512-col granules: the very first matmul
            # needs only tile_q[:, 0:512] plus x0's first chunk
            nc.gpsimd.dma_start(tile_q[:, 0:512], q_ap[:, 0:512])
            x01_chunk(xt0, 0, 0)
            nc.gpsimd.dma_start(tile_q[:, 512:1024], q_ap[:, 512:1024])
            x01_chunk(xt1, 1, 0)
            nc.gpsimd.dma_start(tile_q[:, 1024:2048], q_ap[:, 1024:2048])
            q_dma(1, nc.gpsimd)
            q_dma(2, nc.gpsimd)
            x01_chunk(xt0, 0, 1); x01_chunk(xt1, 1, 1)
            q_dma(3, nc.gpsimd)
            q_dma(4, nc.gpsimd)
            x01_chunk(xt0, 0, 2); x01_chunk(xt1, 1, 2)
            q_dma(5, nc.gpsimd)
            x01_chunk(xt0, 0, 3); x01_chunk(xt1, 1, 3)
            xt2 = xs_dma(2, 1, nc.gpsimd)
            xt3 = xs_dma(3, 1, nc.gpsimd)

            # ---- main loop: out[m, n] = sum_k x[m,k] q[n,k] -----------
            def mm_pair(ps2, xt, base, ko):
                nc.tensor.matmul(
                    ps2[0][:], lhsT=xt[:, base + ko * P:base + (ko + 1) * P],
                    rhs=tile_q[:, ko * NS:ko * NS + 512],
                    start=(ko == 0), stop=(ko == KO - 1))
                nc.tensor.matmul(
                    ps2[1][:], lhsT=xt[:, base + ko * P:base + (ko + 1) * P],
                    rhs=tile_q[:, ko * NS + 512:(ko + 1) * NS],
                    start=(ko == 0), stop=(ko == KO - 1))

            def drain_store(mt, ps2):
                ot = opool.tile([P, NS], BF16, name=f"o_{mt}", tag="o")
                nc.vector.tensor_scalar(
                    ot[:, 0:512], ps2[0][:], 1.0, None, mybir.AluOpType.mult)
                nc.vector.tensor_scalar(
                    ot[:, 512:1024], ps2[1][:], 1.0, None,
                    mybir.AluOpType.mult)
                nc.sync.dma_start(o_ap[mt * P:(mt + 1) * P, :], ot[:])

            def ps_pair(mt):
                return (psum_o.tile([P, 512], F32, name=f"psA_{mt}", tag="ps"),
                        psum_o.tile([P, 512], F32, name=f"psB_{mt}", tag="ps"))

            # m-tiles 0/1 interleaved ko-major (see header)
            ps0, ps1 = ps_pair(0), ps_pair(1)
            for ko in range(KO):
                mm_pair(ps0, xt0, 0, ko)
                mm_pair(ps1, xt1, 0, ko)
            drain_store(0, ps0)
            drain_store(1, ps1)

            for mt, xt in ((2, xt2), (3, xt3)):
                ps = ps_pair(mt)
                for ko in range(KO):
                    mm_pair(ps, xt, 0, ko)
                drain_store(mt, ps)

            # steady state: 1 MiB pair DMAs; both m-tiles of a pair run
            # ko-major interleaved (4 psum banks) so there is only one
            # semaphore-check hiccup (~50ns) per pair instead of two
            for pair in range(2, NPAIR - 1):
                xt = xppool.tile([P, 2 * K], BF16, name=f"xp_{pair}",
                                 tag="xp")
                nc.gpsimd.dma_start(xt[:], x_ap[pair * P:(pair + 1) * P, :])
                psj = (ps_pair(2 * pair), ps_pair(2 * pair + 1))
                for ko in range(KO):
                    mm_pair(psj[0], xt, 0, ko)
                    mm_pair(psj[1], xt, K, ko)
                drain_store(2 * pair, psj[0])
                drain_store(2 * pair + 1, psj[1])

            # last pair: sequential m-tiles, and the final m-tile runs
            # its two psum sweeps back to back so the first drain+store
            # overlaps the second sweep (shorter tail)
            for pair in range(NPAIR - 1, NPAIR):
                xt = xppool.tile([P, 2 * K], BF16, name=f"xp_{pair}",
                                 tag="xp")
                nc.gpsimd.dma_start(xt[:], x_ap[pair * P:(pair + 1) * P, :])
                for j in range(2):
                    mt = 2 * pair + j
                    if mt < MT - 1:
                        ps = ps_pair(mt)
                        for ko in range(KO):
                            mm_pair(ps, xt, j * K, ko)
                        drain_store(mt, ps)
                    else:
                        # last m-tile: sequential psum sweeps so the
                        # first drain+store overlaps the second sweep
                        psA, psB = ps_pair(mt)
                        ot = opool.tile([P, NS], BF16, name=f"o_{mt}",
                                        tag="o")
                        for ko in range(KO):
                            nc.tensor.matmul(
                                psA[:],
                                lhsT=xt[:, j * K + ko * P:
                                        j * K + (ko + 1) * P],
                                rhs=tile_q[:, ko * NS:ko * NS + 512],
                                start=(ko == 0), stop=(ko == KO - 1))
                        nc.vector.tensor_scalar(
                            ot[:, 0:512], psA[:], 1.0, None,
                            mybir.AluOpType.mult)
                        nc.sync.dma_start(
                            o_ap[mt * P:(mt + 1) * P, 0:512], ot[:, 0:512])
                        for ko in range(KO):
                            nc.tensor.matmul(
                                psB[:],
                                lhsT=xt[:, j * K + ko * P:
                                        j * K + (ko + 1) * P],
                                rhs=tile_q[:, ko * NS + 512:(ko + 1) * NS],
                                start=(ko == 0), stop=(ko == KO - 1))
                        nc.vector.tensor_scalar(
                            ot[:, 512:1024], psB[:], 1.0, None,
                            mybir.AluOpType.mult)
                        nc.sync.dma_start(
                            o_ap[mt * P:(mt + 1) * P, 512:1024],
                            ot[:, 512:1024])

    nc.compile()
    return nc


_NC_CACHE = None


def get_nc():
    global _NC_CACHE
    if _NC_CACHE is None:
        _NC_CACHE = build_nc()
    return _NC_CACHE


def make_in_maps(x, weight):
    x2 = np.asarray(x, dtype=np.float32).reshape(M, K)
    w = np.asarray(weight, dtype=np.float32)

    # exact reference prep: scale from the full W, ternary q
    scale = np.float32(1e-4) + np.abs(w).mean(dtype=np.float32)
    q = np.clip(np.rint(w / scale), -1.0, 1.0).astype(np.float32)

    # xdev[pair*128+p, j*2048 + ko*128+m] = scale*x[(2*pair+j)*128+m, ko*128+p]
    xs = (x2 * scale).reshape(NPAIR, 2, P, KO, P)  # [pair, j, m, ko, p]
    xdev = np.ascontiguousarray(
        xs.transpose(0, 4, 1, 3, 2).reshape(M // 2, 2 * K).astype(BF16_NP))

    # qdev_c[p, ko*1024+n] = q[c*1024+n, ko*128+p]  (ternary: exact in fp8)
    q4 = q.reshape(NCORES, NS, KO, P).transpose(0, 3, 2, 1)  # [c, p, ko, n]
    qdev = np.ascontiguousarray(q4.reshape(NCORES, P, KO * NS).astype(FP8_NP))

    return [{"x": xdev, "q": qdev[c]} for c in range(NCORES)]


def kernel(x, weight):
    nc = get_nc()
    in_maps = make_in_maps(x, weight)
    try:
        res = run_bass_kernel_spmd(nc, in_maps, list(range(NCORES)))
    except Exception:
        # transient device errors have been observed on first touch; retry once
        res = run_bass_kernel_spmd(nc, in_maps, list(range(NCORES)))
    out = np.concatenate(
        [np.asarray(res.results[c]["out"]) for c in range(NCORES)], axis=1)
    return np.ascontiguousarray(out, dtype=np.float32).reshape(4, 2048, N_FULL)


# revision 28
# speedup vs baseline: 1.0236x; 1.0021x over previous
"""BitNet linear layer (b1.58-style) on 8 Trainium2 NeuronCores.

Computes: scale = 1e-4 + mean(|W|); q = clip(round(W/scale), -1, 1);
          out = scale * (x @ q.T)
for x [4, 2048, 2048] f32 and W [8192, 2048] f32.

Sharding: tensor-parallel over out_features. Each core gets the full x
(replicated) and a 1024-row shard of the ternary q; cores run fully
independently and the host concatenates the per-core [8192, 1024]
output slices along the feature axis.

The elementwise prep runs once on the host (it is ~0.1% of the FLOPs
and would otherwise be redundantly recomputed per core): the exact
global scale and ternary q (bit-identical rounding vs the reference),
the f32->bf16 casts, and the transposes into SBUF-ready layouts.
`scale` is folded into the bf16 x cast, which is free in accuracy
terms (a single bf16 rounding either way), so the device applies no
scale at all. Remaining error is the bf16 rounding of x plus the bf16
output store (~2.2e-3 measured).

The device is then a pure gap-free bf16 matmul at the PE roofline:
2048 matmuls of N=512 at the 216 ns issue floor ~= 443 us, everything
else hidden behind it. Startup schedule (the only nontrivial part —
the SBUF-write fabric and per-queue DMA rates bound how fast q and the
first x tiles can land):

  - q ships AND stays fp8 (ternary is exact in fp8; 2 MiB instead of
    4, half the SBUF footprint): the matmul takes a bf16 stationary x
    against an fp8 moving q at the same 216 ns/MM cadence — the PE
    upconverts both operands internally. (An earlier "fp8 is 20%
    slower" measurement was a chip-wide P0 2.0 GHz power-state window,
    not an fp8 property; always classify runs by their steady MM
    issue delta before comparing.)
  - The whole startup burst (q slices + the first x tiles, in strict
    need order) rides the gpsimd/SWDGE queue, which is ~2x faster
    than the HWDGE queues for these strided 128-partition tiles.
    One queue in need order is also a natural throttle: later
    prefetches cannot steal bandwidth from the critical first slices
    (a 3-tile-deep x prefetch on its own queue measurably starved q).
    q6/q7 go to the sync queue, idle until the first out-store.
    Every queue shows a ~12-13 us first-completion floor regardless
    of transfer size, so real work is data-bound to start ~14 us in.
  - m-tiles 0 and 1 are interleaved ko-major so each q slice is
    consumed twice per arrival — without this the PE stalls ~5-8 us
    on q arrival even on the fast queue.
  - ~17 dummy matmuls on a zeroed SBUF tile (output never read) run
    during the preamble to carry the PE through the HAM SHORT window
    and up to the data floor, so real matmuls start at 2.4 GHz (the
    idle/cold default is 1.2 GHz).
  - Steady-state x: 1 MiB two-m-tile pair DMAs, also on gpsimd,
    emitted after the burst; the two m-tiles of a pair run ko-major
    interleaved (4 psum banks) so there is one semaphore-check hiccup
    (~50 ns) per pair instead of two.
  - Per m-tile: 16 k-steps of two 512-col accumulating matmuls into a
    psum bank pair (8 banks -> 4 m-tiles in flight), DVE drains psum
    -> bf16 out tile, stores on the sync queue in natural [M, N-shard]
    orientation. The final m-tile runs its two psum sweeps
    sequentially so the first drain+store overlaps the second sweep.
"""

import os
import sys

sys.path.insert(0, "/opt/trn_rl_repo")
os.environ.setdefault("JAX_PLATFORMS", "axon")

import numpy as np
import ml_dtypes

import concourse.bass as bass
import concourse.tile as tile
from concourse import bacc, mybir
from concourse.bass_utils import run_bass_kernel_spmd

F32 = mybir.dt.float32
BF16 = mybir.dt.bfloat16
FP8 = mybir.dt.float8e4
U32 = mybir.dt.uint32
BF16_NP = ml_dtypes.bfloat16
FP8_NP = ml_dtypes.float8_e4m3

NCORES = 8
M = 8192          # tokens (4*2048)
K = 2048          # in_features
N_FULL = 8192     # out_features
NS = N_FULL // NCORES  # 1024 per-core shard
P = 128
KO = K // P       # 16 k-tiles
MT = M // P       # 64 m-tiles
NPAIR = MT // 2   # x pair-DMA rows


def build_nc():
    nc = bacc.Bacc("TRN2", target_bir_lowering=False, debug=False,
                   num_devices=NCORES)
    # x rows pair*128+p hold [j, ko*128+m] -> scale*x[(2*pair+j)*128+m, ko*128+p]
    x_d = nc.dram_tensor("x", [M // 2, 2 * K], BF16, kind="ExternalInput")
    q_d = nc.dram_tensor("q", [P, KO * NS], FP8, kind="ExternalInput")
    o_d = nc.dram_tensor("out", [M, NS], BF16, kind="ExternalOutput")
    x_ap, q_ap, o_ap = x_d.ap(), q_d.ap(), o_d.ap()

    with tile.TileContext(nc) as tc:
        with (
            tc.tile_pool(name="qpool", bufs=1) as qpool,
            tc.tile_pool(name="xspool", bufs=4) as xspool,
            tc.tile_pool(name="xppool", bufs=3) as xppool,
            tc.tile_pool(name="opool", bufs=4) as opool,
            tc.tile_pool(name="psum_o", bufs=8, space="PSUM") as psum_o,
        ):
            # q stays fp8 in SBUF: the ternary weights are exact in fp8,
            # the PE upconverts both operands internally, and an fp8
            # moving operand streams at the same 216 ns/MM as bf16 (an
            # earlier 259 ns measurement was a chip P0 2.0 GHz window,
            # not an fp8 property). Half the SBUF footprint and half the
            # startup-burst bytes, and no expansion ops at all.
            tile_q = qpool.tile([P, KO * NS], FP8, name="q")
            warm = qpool.tile([P, 640], BF16, name="warm")

            # ---- PE warmup (HAM) --------------------------------------
            wz = warm[:].bitcast(U32)
            nc.vector.tensor_scalar(wz, wz, 0, None,
                                    mybir.AluOpType.bitwise_and)
            # dummies bridge the PE from the preamble (~7us) to when the
            # startup burst lands the first q+x slices (~10us): staying
            # busy keeps HAM at 2.4 GHz for the real matmuls.
            psW = psum_o.tile([P, 512], F32, name="psW", tag="ps")
            for _ in range(17):
                nc.tensor.matmul(psW[:], lhsT=warm[:, 0:P],
                                 rhs=warm[:, P:640], start=True, stop=True)

            # ---- startup burst ----------------------------------------
            # Everything rides the gpsimd/SWDGE queue (the fastest for
            # these strided 128-partition tiles) in strict need order —
            # program order on one queue is also a natural throttle, so
            # no later prefetch can steal bandwidth from the critical
            # first slices. q6/q7 go to the sync queue (idle until the
            # first out-store at ~29us): slow but early enough. Every
            # queue shows a ~12-13us first-completion floor regardless
            # of size, so T0 ~= 14us is the data-bound start.
            def q_dma(g, eng):
                eng.dma_start(
                    tile_q[:, 2 * g * NS:2 * (g + 1) * NS],
                    q_ap[:, 2 * g * NS:2 * (g + 1) * NS])

            # x singles for m-tiles 0-3 (pair row mt//2, column half mt%2)
            def xs_dma(mt, chunks, eng):
                xt = xspool.tile([P, K], BF16, name=f"x_{mt}", tag="x")
                r0, c0 = (mt // 2) * P, (mt % 2) * K
                step = K // chunks
                for c in range(chunks):
                    eng.dma_start(
                        xt[:, c * step:(c + 1) * step],
                        x_ap[r0:r0 + P, c0 + c * step:c0 + (c + 1) * step])
                return xt

            # x0/x1 chunks woven between gpsimd q slices in need order
            xt0 = xspool.tile([P, K], BF16, name="x_0", tag="x")
            xt1 = xspool.tile([P, K], BF16, name="x_1", tag="x")
            def x01_chunk(xt, mt, c):
                nc.gpsimd.dma_start(
                    xt[:, c * 512:(c + 1) * 512],
                    x_ap[0:P, mt * K + c * 512:mt * K + (c + 1) * 512])

            q_dma(6, nc.sync)
            q_dma(7, nc.sync)
            # first slice in 512-col granules: the very first matmul
            # needs only tile_q[:, 0:512] plus x0's first chunk
            nc.gpsimd.dma_start(tile_q[:, 0:512], q_ap[:, 0:512])
            x01_chunk(xt0, 0, 0)
            nc.gpsimd.dma_start(tile_q[:, 512:1024], q_ap[:, 512:1024])
            x01_chunk(xt1, 1, 0)
            nc.gpsimd.dma_start(tile_q[:, 1024:2048], q_ap[:, 1024:2048])
            q_dma(1, nc.gpsimd)
            q_dma(2, nc.gpsimd)
            x01_chunk(xt0, 0, 1); x01_chunk(xt1, 1, 1)
            q_dma(3, nc.gpsimd)
            q_dma(4, nc.gpsimd)
            x01_chunk(xt0, 0, 2); x01_chunk(xt1, 1, 2)
            q_dma(5, nc.gpsimd)
            x01_chunk(xt0, 0, 3); x01_chunk(xt1, 1, 3)
            xt2 = xs_dma(2, 1, nc.gpsimd)
            xt3 = xs_dma(3, 1, nc.gpsimd)

            # ---- main loop: out[m, n] = sum_k x[m,k] q[n,k] -----------
            def mm_pair(ps2, xt, base, ko):
                nc.tensor.matmul(
                    ps2[0][:], lhsT=xt[:, base + ko * P:base + (ko + 1) * P],
                    rhs=tile_q[:, ko * NS:ko * NS + 512],
                    start=(ko == 0), stop=(ko == KO - 1))
                nc.tensor.matmul(
                    ps2[1][:], lhsT=xt[:, base + ko * P:base + (ko + 1) * P],
                    rhs=tile_q[:, ko * NS + 512:(ko + 1) * NS],
                    start=(ko == 0), stop=(ko == KO - 1))

            def drain_store(mt, ps2):
                ot = opool.tile([P, NS], BF16, name=f"o_{mt}", tag="o")
                nc.vector.tensor_scalar(
                    ot[:, 0:512], ps2[0][:], 1.0, None, mybir.AluOpType.mult)
                nc.vector.tensor_scalar(
                    ot[:, 512:1024], ps2[1][:], 1.0, None,
                    mybir.AluOpType.mult)
                nc.sync.dma_start(o_ap[mt * P:(mt + 1) * P, :], ot[:])

            def ps_pair(mt):
                return (psum_o.tile([P, 512], F32, name=f"psA_{mt}", tag="ps"),
                        psum_o.tile([P, 512], F32, name=f"psB_{mt}", tag="ps"))

            # m-tiles 0/1 interleaved ko-major (see header)
            ps0, ps1 = ps_pair(0), ps_pair(1)
            for ko in range(KO):
                mm_pair(ps0, xt0, 0, ko)
                mm_pair(ps1, xt1, 0, ko)
            drain_store(0, ps0)
            drain_store(1, ps1)

            for mt, xt in ((2, xt2), (3, xt3)):
                ps = ps_pair(mt)
                for ko in range(KO):
                    mm_pair(ps, xt, 0, ko)
                drain_store(mt, ps)

            # steady state: 1 MiB pair DMAs; both m-tiles of a pair run
            # ko-major interleaved (4 psum banks) so there is only one
            # semaphore-check hiccup (~50ns) per pair instead of two
            for pair in range(2, NPAIR - 1):
                xt = xppool.tile([P, 2 * K], BF16, name=f"xp_{pair}",
                                 tag="xp")
                nc.gpsimd.dma_start(xt[:], x_ap[pair * P:(pair + 1) * P, :])
                psj = (ps_pair(2 * pair), ps_pair(2 * pair + 1))
                for ko in range(KO):
                    mm_pair(psj[0], xt, 0, ko)
                    mm_pair(psj[1], xt, K, ko)
                drain_store(2 * pair, psj[0])
                drain_store(2 * pair + 1, psj[1])

            # last pair: sequential m-tiles, and the final m-tile runs
            # its two psum sweeps back to back so the first drain+store
            # overlaps the second sweep (shorter tail)
            for pair in range(NPAIR - 1, NPAIR):
                xt = xppool.tile([P, 2 * K], BF16, name=f"xp_{pair}",
                                 tag="xp")
                nc.gpsimd.dma_start(xt[:], x_ap[pair * P:(pair + 1) * P, :])
                for j in range(2):
                    mt = 2 * pair + j
                    if mt < MT - 1:
                        ps = ps_pair(mt)
                        for ko in range(KO):
                            mm_pair(ps, xt, j * K, ko)
                        drain_store(mt, ps)
                    else:
                        # last m-tile: sequential psum sweeps so the
                        # first drain+store overlaps the second sweep
                        psA, psB = ps_pair(mt)
                        ot = opool.tile([P, NS], BF16, name=f"o_{mt}",
                                        tag="o")
                        for ko in range(KO):
                            nc.tensor.matmul(
                                psA[:],
                                lhsT=xt[:, j * K + ko * P:
                                        j * K + (ko + 1) * P],
                                rhs=tile_q[:, ko * NS:ko * NS + 512],
                                start=(ko == 0), stop=(ko == KO - 1))
                        nc.vector.tensor_scalar(
                            ot[:, 0:512], psA[:], 1.0, None,
                            mybir.AluOpType.mult)
                        nc.sync.dma_start(
                            o_ap[mt * P:(mt + 1) * P, 0:512], ot[:, 0:512])
                        for ko in range(KO):
                            nc.tensor.matmul(
                                psB[:],
                                lhsT=xt[:, j * K + ko * P:
                                        j * K + (ko + 1) * P],
                                rhs=tile_q[:, ko * NS + 512:(ko + 1) * NS],
                                start=(ko == 0), stop=(ko == KO - 1))
                        nc.vector.tensor_scalar(
                            ot[:, 512:1024], psB[:], 1.0, None,
                            mybir.AluOpType.mult)
                        nc.sync.dma_start(
                            o_ap[mt * P:(mt + 1) * P, 512:1024],
                            ot[:, 512:1024])

    nc.compile()
    return nc


_NC_CACHE = None


def get_nc():
    global _NC_CACHE
    if _NC_CACHE is None:
        _NC_CACHE = build_nc()
    return _NC_CACHE


def make_in_maps(x, weight):
    x2 = np.asarray(x, dtype=np.float32).reshape(M, K)
    w = np.asarray(weight, dtype=np.float32)

    # exact reference prep: scale from the full W, ternary q
    scale = np.float32(1e-4) + np.abs(w).mean(dtype=np.float32)
    q = np.clip(np.rint(w / scale), -1.0, 1.0).astype(np.float32)

    # xdev[pair*128+p, j*2048 + ko*128+m] = scale*x[(2*pair+j)*128+m, ko*128+p]
    xs = (x2 * scale).reshape(NPAIR, 2, P, KO, P)  # [pair, j, m, ko, p]
    xdev = np.ascontiguousarray(
        xs.transpose(0, 4, 1, 3, 2).reshape(M // 2, 2 * K).astype(BF16_NP))

    # qdev_c[p, ko*1024+n] = q[c*1024+n, ko*128+p]  (ternary: exact in fp8)
    q4 = q.reshape(NCORES, NS, KO, P).transpose(0, 3, 2, 1)  # [c, p, ko, n]
    qdev = np.ascontiguousarray(q4.reshape(NCORES, P, KO * NS).astype(FP8_NP))

    return [{"x": xdev, "q": qdev[c]} for c in range(NCORES)]


def kernel(x, weight):
    nc = get_nc()
    in_maps = make_in_maps(x, weight)
    try:
        res = run_bass_kernel_spmd(nc, in_maps, list(range(NCORES)))
    except Exception:
        # transient device errors have been observed on first touch; retry once
        res = run_bass_kernel_spmd(nc, in_maps, list(range(NCORES)))
    out = np.concatenate(
        [np.asarray(res.results[c]["out"]) for c in range(NCORES)], axis=1)
    return np.ascontiguousarray(out, dtype=np.float32).reshape(4, 2048, N_FULL)


# revision 33
# speedup vs baseline: 1.3411x; 1.3101x over previous
"""BitNet linear layer (b1.58-style) on 8 Trainium2 NeuronCores.

Computes: scale = 1e-4 + mean(|W|); q = clip(round(W/scale), -1, 1);
          out = scale * (x @ q.T)
for x [4, 2048, 2048] f32 and W [8192, 2048] f32.

Sharding: tensor-parallel over out_features. Each core gets the full x
(replicated) and a 1024-row shard of the ternary q; cores run fully
independently and the host concatenates the per-core [8192, 1024]
output slices along the feature axis.

The elementwise prep runs once on the host (it is ~0.1% of the FLOPs
and would otherwise be redundantly recomputed per core): the exact
global scale and ternary q (bit-identical rounding vs the reference),
the narrowing casts, and the transposes into SBUF-ready layouts. The
scale is applied on-device during the psum drain (a [128,1] f32 input)
because the two precision halves below must share one accumulator.

Mixed-precision contraction — the core idea. The bf16 matmul issue
floor is 216 ns per N=512 matmul (1 moving col/cycle at 2.4 GHz), i.e.
443 us for the full 2048-deep contraction. fp8 DoubleRow mode packs 2
weights per PE cell and streams 2 fp8 moving elements/cycle: measured
216 ns for a contraction-256 N=512 matmul — a genuine 2x. Full fp8 is
too lossy for the 2e-2 gate (e4m3 on x alone gives ~2.5% out error),
but a *split* contraction works: k-tiles 0-7 run bf16 x (exact-ish),
k-tiles 8-15 run fp8 x in DoubleRow pairs, both accumulating into the
same psum bank. Measured rel err on the reference inputs: 1.70e-2
(15% inside the gate, deterministic); stream time per m-tile drops
from 32x216 to 24x216 ns = 25% fewer PE cycles.

DoubleRow semantics (validated on HW, exact): lhsT [p, i, m] fp8,
rhs [p, i, n] fp8 (i = 0,1 the packed pair), out[m, n] +=
sum_p sum_i lhsT[p,i,m] * rhs[p,i,n]; the host lays out both operands
with the same (p, i) -> k mapping: k = 1024 + kp*256 + i*128 + p.

Startup/steady schedule (bounded by per-queue DMA rates, the ~12-13 us
first-completion floor every queue shows regardless of size, and the
shared HBM/SBUF-write fabric):
  - The critical startup set is split across three queues to race
    three first-completion floors in parallel: q slices on gpsimd
    (SWDGE, ~2x faster than HWDGE for these strided 128-partition
    tiles), x0/x1 bf16 chunks on scalar, late q slices on sync.
  - m-tiles 0 and 1 run ko-major interleaved so each q slice is
    consumed twice per arrival; steady-state pairs likewise (and one
    ~50 ns semaphore hiccup per pair instead of two).
  - ~13 dummy matmuls on a zeroed SBUF tile (output never read) carry
    the PE through the HAM SHORT window up to the data floor so real
    matmuls start at 2.4 GHz (idle default is 1.2 GHz).
  - Steady-state x rides gpsimd as two-m-tile pair DMAs (bf16 and fp8
    parts), emitted after the burst in program order — a natural
    throttle that keeps prefetch from starving the startup burst.
  - Per m-tile: 8 bf16 k-steps + 4 DoubleRow k-pair-steps of two
    512-col matmuls into a psum bank pair; DVE drains psum * scale ->
    bf16 out tile; stores on sync in natural [M, N-shard] orientation.
    The final m-tile runs sweeps of shrinking width (512/256/256) so
    the last serial chain is one 256-col drain + 64 KiB store.
"""

import os
import sys

sys.path.insert(0, "/opt/trn_rl_repo")
os.environ.setdefault("JAX_PLATFORMS", "axon")

import numpy as np
import ml_dtypes

import concourse.bass as bass
import concourse.tile as tile
from concourse import bacc, mybir
from concourse.bass_utils import run_bass_kernel_spmd

F32 = mybir.dt.float32
BF16 = mybir.dt.bfloat16
FP8 = mybir.dt.float8e4
U32 = mybir.dt.uint32
BF16_NP = ml_dtypes.bfloat16
FP8_NP = ml_dtypes.float8_e4m3
DR = mybir.MatmulPerfMode.DoubleRow

NCORES = 8
M = 8192          # tokens (4*2048)
K = 2048          # in_features
N_FULL = 8192     # out_features
NS = N_FULL // NCORES  # 1024 per-core shard
P = 128
KB = 8            # bf16 k-tiles (k < 1024)
KP = 4            # fp8 DoubleRow k-pairs (k >= 1024, 256 each)
MT = M // P       # 64 m-tiles
NPAIR = MT // 2
XBW = KB * P      # bf16 x cols per m-tile (1024)
X8W = KP * 2 * P  # fp8 x cols per m-tile (1024)


def build_nc():
    nc = bacc.Bacc("TRN2", target_bir_lowering=False, debug=False,
                   num_devices=NCORES)
    # xb rows pair*128+p: [j, ko*128+m] -> x[(2*pair+j)*128+m, ko*128+p]
    xb_d = nc.dram_tensor("xb", [M // 2, 2 * XBW], BF16, kind="ExternalInput")
    # x8 rows pair*128+p: [j, kp, i, m] -> x[(2*pair+j)*128+m, 1024+kp*256+i*128+p]
    x8_d = nc.dram_tensor("x8", [M // 2, 2 * X8W], FP8, kind="ExternalInput")
    # qa[p, ko*1024+n] = q[n, ko*128+p]            (bf16-half moving operand)
    qa_d = nc.dram_tensor("qa", [P, KB * NS], FP8, kind="ExternalInput")
    # qd[p, (kp, i, n)] = q[n, 1024+kp*256+i*128+p] (DoubleRow moving operand)
    qd_d = nc.dram_tensor("qd", [P, KP * 2 * NS], FP8, kind="ExternalInput")
    sc_d = nc.dram_tensor("sc", [P, 1], F32, kind="ExternalInput")
    o_d = nc.dram_tensor("out", [M, NS], BF16, kind="ExternalOutput")
    xb_ap, x8_ap = xb_d.ap(), x8_d.ap()
    qa_ap, qd_ap, sc_ap, o_ap = qa_d.ap(), qd_d.ap(), sc_d.ap(), o_d.ap()

    with tile.TileContext(nc) as tc:
        with (
            tc.tile_pool(name="qpool", bufs=1) as qpool,
            tc.tile_pool(name="xspool", bufs=4) as xspool,
            tc.tile_pool(name="x8spool", bufs=4) as x8spool,
            tc.tile_pool(name="xbpool", bufs=3) as xbpool,
            tc.tile_pool(name="x8pool", bufs=3) as x8pool,
            tc.tile_pool(name="opool", bufs=4) as opool,
            tc.tile_pool(name="psum_o", bufs=8, space="PSUM") as psum_o,
        ):
            tile_qa = qpool.tile([P, KB * NS], FP8, name="qa")
            tile_qd = qpool.tile([P, KP, 2, NS], FP8, name="qd")
            sc = qpool.tile([P, 1], F32, name="sc")
            warm = qpool.tile([P, 640], BF16, name="warm")

            # ---- PE warmup (HAM) --------------------------------------
            wz = warm[:].bitcast(U32)
            nc.vector.tensor_scalar(wz, wz, 0, None,
                                    mybir.AluOpType.bitwise_and)
            psW = psum_o.tile([P, 512], F32, name="psW", tag="ps")
            for _ in range(13):
                nc.tensor.matmul(psW[:], lhsT=warm[:, 0:P],
                                 rhs=warm[:, P:640], start=True, stop=True)

            # ---- startup burst ----------------------------------------
            nc.sync.dma_start(sc[:], sc_ap[:, :])
            # late-needed q: DoubleRow half on sync (slow queue, but the
            # fp8 k-steps only start ~7us into each m-tile sweep)
            for g in range(KP):
                nc.sync.dma_start(tile_qd[:, g, :, :],
                                  qd_ap[:, g * 2 * NS:(g + 1) * 2 * NS])

            # x0/x1 bf16 chunks on the otherwise-idle scalar queue
            xt0 = xspool.tile([P, XBW], BF16, name="xb_0", tag="x")
            xt1 = xspool.tile([P, XBW], BF16, name="xb_1", tag="x")
            def x01_chunk(xt, mt, c):
                nc.scalar.dma_start(
                    xt[:, c * 512:(c + 1) * 512],
                    xb_ap[0:P, mt * XBW + c * 512:mt * XBW + (c + 1) * 512])
            for c in range(2):
                x01_chunk(xt0, 0, c); x01_chunk(xt1, 1, c)
            # fp8 x halves for mt0/1 (needed from k-step 8, ~7us in)
            x80 = x8spool.tile([P, 1, KP, 2, P], FP8, name="x8_0", tag="x8")
            x81 = x8spool.tile([P, 1, KP, 2, P], FP8, name="x8_1", tag="x8")
            nc.scalar.dma_start(x80[:], x8_ap[0:P, 0:X8W])
            nc.scalar.dma_start(x81[:], x8_ap[0:P, X8W:2 * X8W])

            # critical bf16-half q on gpsimd, first slice in 512-granules
            nc.gpsimd.dma_start(tile_qa[:, 0:512], qa_ap[:, 0:512])
            nc.gpsimd.dma_start(tile_qa[:, 512:1024], qa_ap[:, 512:1024])
            nc.gpsimd.dma_start(tile_qa[:, 1024:2048], qa_ap[:, 1024:2048])
            for g in range(1, KB // 2):
                nc.gpsimd.dma_start(
                    tile_qa[:, 2 * g * NS:2 * (g + 1) * NS],
                    qa_ap[:, 2 * g * NS:2 * (g + 1) * NS])

            # x singles for m-tiles 2-3
            def xs_single(mt):
                xt = xspool.tile([P, XBW], BF16, name=f"xb_{mt}", tag="x")
                nc.gpsimd.dma_start(
                    xt[:], xb_ap[(mt // 2) * P:(mt // 2 + 1) * P,
                                 (mt % 2) * XBW:(mt % 2 + 1) * XBW])
                x8t = x8spool.tile([P, 1, KP, 2, P], FP8, name=f"x8_{mt}",
                                   tag="x8")
                nc.gpsimd.dma_start(
                    x8t[:], x8_ap[(mt // 2) * P:(mt // 2 + 1) * P,
                                  (mt % 2) * X8W:(mt % 2 + 1) * X8W])
                return xt, x8t
            xt2, x82 = xs_single(2)
            xt3, x83 = xs_single(3)

            # ---- main loop: out[m, n] = sum_k x[m,k] q[n,k] -----------
            def mm_bf16(ps2, xt, base, ko):
                lhsT = xt[:, base + ko * P:base + (ko + 1) * P]
                nc.tensor.matmul(
                    ps2[0][:], lhsT=lhsT,
                    rhs=tile_qa[:, ko * NS:ko * NS + 512],
                    start=(ko == 0), stop=False)
                nc.tensor.matmul(
                    ps2[1][:], lhsT=lhsT,
                    rhs=tile_qa[:, ko * NS + 512:(ko + 1) * NS],
                    start=(ko == 0), stop=False)

            def mm_dr(ps2, x8t, j8, kp):
                lhsT = x8t[:, j8, kp, :, :]
                nc.tensor.matmul(
                    ps2[0][:], lhsT=lhsT, rhs=tile_qd[:, kp, :, 0:512],
                    start=False, stop=(kp == KP - 1), perf_mode=DR)
                nc.tensor.matmul(
                    ps2[1][:], lhsT=lhsT, rhs=tile_qd[:, kp, :, 512:NS],
                    start=False, stop=(kp == KP - 1), perf_mode=DR)

            def drain_store(mt, ps2):
                ot = opool.tile([P, NS], BF16, name=f"o_{mt}", tag="o")
                nc.vector.tensor_scalar(
                    ot[:, 0:512], ps2[0][:], sc[:], None,
                    mybir.AluOpType.mult)
                nc.vector.tensor_scalar(
                    ot[:, 512:1024], ps2[1][:], sc[:], None,
                    mybir.AluOpType.mult)
                nc.sync.dma_start(o_ap[mt * P:(mt + 1) * P, :], ot[:])

            def ps_pair(mt):
                return (psum_o.tile([P, 512], F32, name=f"psA_{mt}", tag="ps"),
                        psum_o.tile([P, 512], F32, name=f"psB_{mt}", tag="ps"))

            def sweep_interleaved(mts, xts, x8ts, bases, bases8):
                pss = [ps_pair(mt) for mt in mts]
                for ko in range(KB):
                    for ps2, xt, b in zip(pss, xts, bases):
                        mm_bf16(ps2, xt, b, ko)
                for kp in range(KP):
                    for ps2, x8t, b8 in zip(pss, x8ts, bases8):
                        mm_dr(ps2, x8t, b8, kp)
                for mt, ps2 in zip(mts, pss):
                    drain_store(mt, ps2)

            # m-tiles 0/1 interleaved (startup), then 2/3
            sweep_interleaved((0, 1), (xt0, xt1), (x80, x81), (0, 0), (0, 0))
            sweep_interleaved((2, 3), (xt2, xt3), (x82, x83), (0, 0), (0, 0))


            # steady state: two-m-tile pair DMAs on gpsimd
            for pair in range(2, NPAIR - 1):
                xbt = xbpool.tile([P, 2 * XBW], BF16, name=f"xbp_{pair}",
                                  tag="xbp")
                nc.gpsimd.dma_start(
                    xbt[:], xb_ap[pair * P:(pair + 1) * P, :])
                x8t = x8pool.tile([P, 2, KP, 2, P], FP8, name=f"x8p_{pair}",
                                  tag="x8p")
                nc.gpsimd.dma_start(
                    x8t[:], x8_ap[pair * P:(pair + 1) * P, :])
                sweep_interleaved(
                    (2 * pair, 2 * pair + 1), (xbt, xbt), (x8t, x8t),
                    (0, XBW), (0, 1))

            # last pair: sequential m-tiles; final m-tile in shrinking
            # widths so the last serial chain is a 256-col drain + 64 KiB
            pair = NPAIR - 1
            xbt = xbpool.tile([P, 2 * XBW], BF16, name=f"xbp_{pair}",
                              tag="xbp")
            nc.gpsimd.dma_start(xbt[:], xb_ap[pair * P:(pair + 1) * P, :])
            x8t = x8pool.tile([P, 2, KP, 2, P], FP8, name=f"x8p_{pair}",
                              tag="x8p")
            nc.gpsimd.dma_start(x8t[:], x8_ap[pair * P:(pair + 1) * P, :])

            mt = MT - 2
            ps = ps_pair(mt)
            for ko in range(KB):
                mm_bf16(ps, xbt, 0, ko)
            for kp in range(KP):
                mm_dr(ps, x8t, 0, kp)
            drain_store(mt, ps)

            mt = MT - 1
            ot = opool.tile([P, NS], BF16, name=f"o_{mt}", tag="o")
            for n0, nw in ((0, 512), (512, 256), (768, 256)):
                ps1 = psum_o.tile([P, 512], F32, name=f"ps_{mt}_{n0}",
                                  tag="ps")
                for ko in range(KB):
                    nc.tensor.matmul(
                        ps1[:, 0:nw],
                        lhsT=xbt[:, XBW + ko * P:XBW + (ko + 1) * P],
                        rhs=tile_qa[:, ko * NS + n0:ko * NS + n0 + nw],
                        start=(ko == 0), stop=False)
                for kp in range(KP):
                    lhsT = x8t[:, 1, kp, :, :]
                    nc.tensor.matmul(
                        ps1[:, 0:nw], lhsT=lhsT,
                        rhs=tile_qd[:, kp, :, n0:n0 + nw],
                        start=False, stop=(kp == KP - 1), perf_mode=DR)
                nc.vector.tensor_scalar(
                    ot[:, n0:n0 + nw], ps1[:, 0:nw], sc[:], None,
                    mybir.AluOpType.mult)
                nc.sync.dma_start(
                    o_ap[mt * P:(mt + 1) * P, n0:n0 + nw], ot[:, n0:n0 + nw])

    nc.compile()
    return nc


_NC_CACHE = None


def get_nc():
    global _NC_CACHE
    if _NC_CACHE is None:
        _NC_CACHE = build_nc()
    return _NC_CACHE


def make_in_maps(x, weight):
    x2 = np.asarray(x, dtype=np.float32).reshape(M, K)
    w = np.asarray(weight, dtype=np.float32)

    # exact reference prep: scale from the full W, ternary q
    scale = np.float32(1e-4) + np.abs(w).mean(dtype=np.float32)
    q = np.clip(np.rint(w / scale), -1.0, 1.0).astype(np.float32)

    # bf16 half: xb[pair*128+p, j*1024 + ko*128+m] = x[(2p+j)*128+m, ko*128+p]
    xlo = x2[:, :XBW].reshape(NPAIR, 2, P, KB, P)   # [pair, j, m, ko, p]
    xb = np.ascontiguousarray(
        xlo.transpose(0, 4, 1, 3, 2).reshape(M // 2, 2 * XBW).astype(BF16_NP))

    # fp8 half: x8[pair*128+p, j, kp, i, m] = x[(2p+j)*128+m, 1024+kp*256+i*128+p]
    xhi = x2[:, XBW:].reshape(NPAIR, 2, P, KP, 2, P)  # [pair, j, m, kp, i, p]
    x8 = np.ascontiguousarray(
        xhi.transpose(0, 5, 1, 3, 4, 2).reshape(M // 2, 2 * X8W).astype(FP8_NP))

    # qa[c, p, ko*1024+n] = q[c*1024+n, ko*128+p]
    qlo = q[:, :XBW].reshape(NCORES, NS, KB, P).transpose(0, 3, 2, 1)
    qa = np.ascontiguousarray(
        qlo.reshape(NCORES, P, KB * NS).astype(FP8_NP))

    # qd[c, p, kp, i, n] = q[c*1024+n, 1024+kp*256+i*128+p]
    qhi = q[:, XBW:].reshape(NCORES, NS, KP, 2, P).transpose(0, 4, 2, 3, 1)
    qd = np.ascontiguousarray(
        qhi.reshape(NCORES, P, KP * 2 * NS).astype(FP8_NP))

    sc = np.full((P, 1), scale, dtype=np.float32)
    return [{"xb": xb, "x8": x8, "qa": qa[c], "qd": qd[c], "sc": sc}
            for c in range(NCORES)]


def kernel(x, weight):
    nc = get_nc()
    in_maps = make_in_maps(x, weight)
    try:
        res = run_bass_kernel_spmd(nc, in_maps, list(range(NCORES)))
    except Exception:
        # transient device errors have been observed on first touch; retry once
        res = run_bass_kernel_spmd(nc, in_maps, list(range(NCORES)))
    out = np.concatenate(
        [np.asarray(res.results[c]["out"]) for c in range(NCORES)], axis=1)
    return np.ascontiguousarray(out, dtype=np.float32).reshape(4, 2048, N_FULL)
